# revision 30
# baseline (speedup 1.0000x reference)
"""Multi-head attention Trainium2 kernel (bs=4, slen=1024, dim=1024, 16 heads).

Sharding: 8 cores = 4 batches x 2 head-groups (8 heads / 512 features each).

v3 design (power-throttle aware, bf16 matmuls):
  - All matmuls bf16 (1 cyc/col on the PE; fp8 DoubleRow was tried and is
    numerically unusable here: the softmax is near-uniform so the context
    is an incoherent average and per-element fp8 noise lands on the output
    at full relative strength, ~3 percent).
  - ScalarE runs ONLY the softmax exps: 64 x [128, 2x512] batched over the
    two heads of a pair sharing a 2-bank PSUM tile, writing bf16 weights,
    plus the 8 partition-crossing ctx stashes nothing else can do.
  - PSUM->SBUF copies on DVE; softmax denominators hop partitions via
    SBUF->SBUF DMA; one DVE reciprocal per seq chunk; 1/den broadcast by a
    selector matmul.
  - Static interleave: the scores stream is padded with proj/v/ctx/outproj
    matmuls so the PE never waits on ScalarE exp latency; out-projection of
    chunk 0 overlaps attention of chunk 1.
  - v bias is dropped on-device; host folds v_b @ out_w.T into the output
    bias (exact under softmax normalization).
Host sums the two head-group partials per batch and adds the biases.
"""

import numpy as np

BS, SLEN, DIM = 4, 1024, 1024
H, DH = 16, 64
P = 128            # partitions
NB = 512           # matmul free-dim chunk (one PSUM bank of fp32)
FPC = 512          # features per core (8 heads)
DT = DIM // P      # 8 contraction tiles over model dim
FT = FPC // P      # 4 feature tiles per core
QC = SLEN // NB    # 2 seq chunks
ST = SLEN // P     # 8 seq tiles
HP = 4             # head pairs per core

_STATE = {}

# set to True by test harness to capture an NTFF profile
TRACE = False
TRACE_KWARGS = {}
LAST_RESULT = None


def _build():
    from contextlib import ExitStack

    import concourse.tile as tile
    from concourse import bacc, mybir

    f32 = mybir.dt.float32
    f32r = mybir.dt.float32r
    bf16 = mybir.dt.bfloat16
    AF = mybir.ActivationFunctionType
    ALU = mybir.AluOpType

    nc = bacc.Bacc("TRN2", target_bir_lowering=False, debug=False)

    xt_d = nc.dram_tensor("xt", [DIM, SLEN], bf16, kind="ExternalInput")
    wqt_d = nc.dram_tensor("wqt", [DIM, FPC], bf16, kind="ExternalInput")
    wkt_d = nc.dram_tensor("wkt", [DIM, FPC], bf16, kind="ExternalInput")
    wvt_d = nc.dram_tensor("wvt", [DIM, FPC], bf16, kind="ExternalInput")
    wot_d = nc.dram_tensor("wot", [FPC, DIM], bf16, kind="ExternalInput")
    qb_d = nc.dram_tensor("qb", [P, FT], f32, kind="ExternalInput")
    kb_d = nc.dram_tensor("kb", [P, FT], f32, kind="ExternalInput")
    negb_d = nc.dram_tensor("negb", [P, ST], f32, kind="ExternalInput")
    sel_d = nc.dram_tensor("sel", [2, P], f32r, kind="ExternalInput")
    out_d = nc.dram_tensor("out", [SLEN, DIM], f32, kind="ExternalOutput")

    with tile.TileContext(nc) as tc:
        with ExitStack() as ctx:
            consts = ctx.enter_context(tc.tile_pool(name="consts", bufs=1))
            big = ctx.enter_context(tc.tile_pool(name="big", bufs=1))
            sm = ctx.enter_context(tc.tile_pool(name="sm", bufs=1))
            psum = ctx.enter_context(tc.tile_pool(name="psum", bufs=1, space="PSUM"))

            # ---- constants (scalar queue; see input DMA block below) ----
            negb_sb = consts.tile([P, ST], f32)
            qb_sb = consts.tile([P, FT], f32)
            kb_sb = consts.tile([P, FT], f32)
            sel_sb = consts.tile([2, P], f32r)

            # ---- big SBUF tensors (inputs as per-slab tiles so the
            # first projection chains ride the DMA stream) ----
            xth = [[big.tile([P, NB], bf16, name=f"xt{t}_{q}")
                    for q in range(QC)] for t in range(DT)]
            wqts = [big.tile([P, FPC], bf16, name=f"wq{t}") for t in range(DT)]
            wkts = [big.tile([P, FPC], bf16, name=f"wk{t}") for t in range(DT)]
            wvts = [big.tile([P, FPC], bf16, name=f"wv{t}") for t in range(DT)]
            wots = [big.tile([P, DIM], bf16, name=f"wo{t}") for t in range(FT)]
            qT_sb = big.tile([P, FT, SLEN], bf16, name="qT")
            kT_sb = big.tile([P, FT, SLEN], bf16, name="kT")
            v_sb = big.tile([P, ST, HP * 2, DH + 1], bf16, name="v")
            wts2 = big.tile([P, ST, 2, NB], bf16, name="wts")
            ctall = big.tile([P, HP * QC, NB], f32, name="ctall")
            ctxn_sb = big.tile([P, HP, SLEN], bf16, name="ctxn")
            # denominator staging at partition 64 (psum-row aligned), then
            # DMA-gathered to partitions 0-7 for the reciprocal
            dnst = big.tile([DH + 1, 2 * HP, NB], f32, name="dnst")
            dnl = [[big.tile([2, NB], f32, name=f"dnl{q}_{h}")
                    for h in range(HP)] for q in range(QC)]
            rtm = [[big.tile([2, NB], f32, name=f"rtm{q}_{h}")
                    for h in range(HP)] for q in range(QC)]
            rca = [[big.tile([2, NB], f32r, name=f"rca{q}_{h}")
                    for h in range(HP)] for q in range(QC)]

            # input DMAs split across both HWDGE issue queues, in
            # consumption order: sync carries x/wq (prologue-critical),
            # scalar carries wk first, then consts, wv, wo
            for t in range(DT):
                nc.sync.dma_start(xth[t][0], xt_d[t * P:(t + 1) * P, 0:NB])
                nc.sync.dma_start(wqts[t], wqt_d[t * P:(t + 1) * P, :])
            for t in range(DT):
                nc.sync.dma_start(xth[t][1], xt_d[t * P:(t + 1) * P, NB:SLEN])
            for t in range(DT):
                nc.scalar.dma_start(wkts[t], wkt_d[t * P:(t + 1) * P, :])
            nc.scalar.dma_start(negb_sb, negb_d[:])
            nc.scalar.dma_start(qb_sb, qb_d[:])
            nc.scalar.dma_start(kb_sb, kb_d[:])
            nc.scalar.dma_start(sel_sb, sel_d[:])
            for t in range(DT):
                nc.scalar.dma_start(wvts[t], wvt_d[t * P:(t + 1) * P, :])
            for t in range(FT):
                nc.scalar.dma_start(wots[t], wot_d[t * P:(t + 1) * P, :])

            # denominator ones-column of v_aug
            nc.vector.memset(v_sb[:, :, :, DH:DH + 1], 1.0)

            # ---------- emission units ----------
            def proj_unit(dst, wt_sb, ft, qc, scale, bias_ap):
                """x.T @ W.T chunk -> dst[:, ft, qc*NB:...]."""
                sl = slice(qc * NB, (qc + 1) * NB)
                ps = psum.tile([P, NB], f32, tag="mmo", bufs=2, name="mmo")
                for t in range(DT):
                    nc.tensor.matmul(
                        ps,
                        lhsT=wt_sb[t][:, ft * P:(ft + 1) * P],
                        rhs=xth[t][qc][:],
                        start=(t == 0), stop=(t == DT - 1))
                nc.vector.tensor_scalar(
                    out=dst[:, ft, sl], in0=ps, scalar1=scale,
                    scalar2=bias_ap, op0=ALU.mult, op1=ALU.add)

            def v_unit(st):
                """v (seq-major) tile st."""
                ps = psum.tile([P, NB], f32, tag="mmo", bufs=2, name="mmo")
                sq, so = divmod(st * P, NB)
                for t in range(DT):
                    nc.tensor.matmul(
                        ps,
                        lhsT=xth[t][sq][:, so:so + P],
                        rhs=wvts[t][:],
                        start=(t == 0), stop=(t == DT - 1))
                nc.vector.tensor_copy(
                    v_sb[:, st, :, 0:DH],
                    ps.rearrange("p (h e) -> p h e", h=HP * 2))

            def sc_unit(qc, hp, kt):
                """scoresT for both heads of pair hp, key tile kt; exp->bf16."""
                sl = slice(qc * NB, (qc + 1) * NB)
                ksl = slice(kt * P, (kt + 1) * P)
                sc = psum.tile([P, 2, NB], f32, tag="sc", bufs=2, name="sc")
                nc.tensor.matmul(
                    sc[:, 0, :], lhsT=kT_sb[0:DH, hp, ksl],
                    rhs=qT_sb[0:DH, hp, sl], tile_position=(0, 0))
                nc.tensor.matmul(
                    sc[:, 1, :], lhsT=kT_sb[DH:P, hp, ksl],
                    rhs=qT_sb[DH:P, hp, sl], tile_position=(DH, 0))
                nc.scalar.activation(
                    wts2[:, kt, :, :], sc, AF.Exp,
                    bias=negb_sb[:, kt:kt + 1], scale=1.0)

            def ctx_mm(pc, hp, a, kt):
                """ctx accumulation for head a of pair hp, key tile kt."""
                nc.tensor.matmul(
                    pc, lhsT=v_sb[:, kt, 2 * hp + a, :],
                    rhs=wts2[:, kt, a, :],
                    start=(kt == 0), stop=(kt == ST - 1))

            def stash_unit(qc, hp, a, pc):
                """ctx rows -> ctall; denominator row -> dnst -> dnl via DMA.

                Head a=0 rows are partition-aligned (DVE); head a=1 rows must
                cross partitions 0-63 -> 64-127, which only ScalarE can do.
                The denominator hops partitions via an SBUF->SBUF DMA.
                """
                j = hp * QC + qc
                if a == 0:
                    nc.vector.tensor_copy(ctall[0:DH, j, :], pc[0:DH, :])
                else:
                    nc.scalar.copy(ctall[DH:P, j, :], pc[0:DH, :])
                r = 2 * hp + a
                nc.vector.tensor_copy(
                    dnst[DH:DH + 1, r, :], pc[DH:DH + 1, :])
                if a == 1:
                    # both heads staged: gather, approx-reciprocal, round
                    # to f32r for the broadcast matmul
                    nc.sync.dma_start(
                        dnl[qc][hp][:], dnst[DH:DH + 1, 2 * hp:2 * hp + 2, :])
                    nc.vector.reciprocal_approx_fast(
                        out=rtm[qc][hp][:], in_=dnl[qc][hp][:])
                    nc.vector.tensor_copy(rca[qc][hp][:], rtm[qc][hp][:])

            def norm_unit(qc, hp):
                """broadcast 1/den via selector matmul; multiply on DVE."""
                sl = slice(qc * NB, (qc + 1) * NB)
                j = hp * QC + qc
                pb = psum.tile([P, NB], f32, tag="mmo", bufs=2, name="mmo")
                nc.tensor.matmul(pb, lhsT=sel_sb[:], rhs=rca[qc][hp][:])
                nc.vector.tensor_mul(ctxn_sb[:, hp, sl], ctall[:, j, :], pb)

            def outproj_unit(qc, qt, jc):
                po = psum.tile([P, NB], f32, tag="mmo", bufs=2, name="mmo")
                for ft in range(FT):
                    nc.tensor.matmul(
                        po, lhsT=ctxn_sb[:, ft, qt * P:(qt + 1) * P],
                        rhs=wots[ft][:, jc * NB:(jc + 1) * NB],
                        start=(ft == 0), stop=(ft == FT - 1))
                ob = sm.tile([P, NB], f32, tag="outsb", bufs=4, name="ob")
                nc.vector.tensor_copy(ob, po)
                eng = nc.scalar if qc == 1 else nc.sync
                eng.dma_start(
                    out_d[qt * P:(qt + 1) * P, jc * NB:(jc + 1) * NB], ob)

            # ---------- static schedule ----------
            def qf(ft, qc):
                return lambda: proj_unit(
                    qT_sb, wqts, ft, qc, 0.125, qb_sb[:, ft:ft + 1])

            def kf(ft, qc):
                return lambda: proj_unit(
                    kT_sb, wkts, ft, qc, 1.0, kb_sb[:, ft:ft + 1])

            # prologue: the bare minimum for the first four score tiles
            with nc.named_scope("prologue"):
                qf(0, 0)()
                kf(0, 0)()

            def fillers_qc0():
                # 12 slots per hp block; constraints: v_kt emitted before
                # hp0's ctx for that kt; kf(0,1) before sc4 of hp0; k ft
                # (both qc) and q(ft, qc0) complete before block ft's first
                # scores; q(ft, qc1) any time before qc1 attention.
                units = [
                    # hp0 slots
                    kf(0, 1), lambda: v_unit(0), lambda: v_unit(1),
                    lambda: v_unit(2), lambda: v_unit(3), lambda: v_unit(4),
                    lambda: v_unit(5), lambda: v_unit(6), lambda: v_unit(7),
                    qf(1, 0), kf(1, 0), kf(1, 1),
                    # hp1 slots
                    qf(2, 0), kf(2, 0), kf(2, 1), qf(0, 1), qf(1, 1),
                    None, None, None, None, None, None, None,
                    # hp2 slots
                    qf(3, 0), kf(3, 0), kf(3, 1), qf(2, 1), None, None,
                    None, None, None, None, None, None,
                    # hp3 slots
                    qf(3, 1),
                ]
                yield from units

            def fillers_qc1():
                for qt in range(ST // QC):
                    for jc in range(QC):
                        yield lambda qt=qt, jc=jc: outproj_unit(0, qt, jc)

            def attention(qc, fillers):
                def slot():
                    f = next(fillers, None)
                    if f is not None:
                        f()

                carry = None  # deferred (ctx kt 5-7 + stash) of previous hp
                for hp in range(HP):
                    with nc.named_scope("attn"):
                        pcA = psum.tile([DH + 1, NB], f32, tag="ctx", bufs=2,
                                        name="pcA")
                        pcB = psum.tile([DH + 1, NB], f32, tag="ctx", bufs=2,
                                        name="pcB")
                        sc_unit(qc, hp, 0)
                        if carry is not None:
                            carry()
                            carry = None
                        sc_unit(qc, hp, 1)
                        slot()
                        sc_unit(qc, hp, 2)
                        slot()
                        sc_unit(qc, hp, 3)
                        slot()
                        ctx_mm(pcA, hp, 0, 0)
                        ctx_mm(pcB, hp, 1, 0)
                        slot()
                        sc_unit(qc, hp, 4)
                        slot()
                        ctx_mm(pcA, hp, 0, 1)
                        ctx_mm(pcB, hp, 1, 1)
                        slot()
                        sc_unit(qc, hp, 5)
                        slot()
                        ctx_mm(pcA, hp, 0, 2)
                        ctx_mm(pcB, hp, 1, 2)
                        slot()
                        sc_unit(qc, hp, 6)
                        slot()
                        ctx_mm(pcA, hp, 0, 3)
                        ctx_mm(pcB, hp, 1, 3)
                        slot()
                        sc_unit(qc, hp, 7)
                        slot()
                        ctx_mm(pcA, hp, 0, 4)
                        ctx_mm(pcB, hp, 1, 4)
                        slot()

                        def finish(qc=qc, hp=hp, pcA=pcA, pcB=pcB):
                            for kt in (5, 6, 7):
                                ctx_mm(pcA, hp, 0, kt)
                                ctx_mm(pcB, hp, 1, kt)
                            stash_unit(qc, hp, 0, pcA)
                            stash_unit(qc, hp, 1, pcB)
                            norm_unit(qc, hp)
                        if hp < HP - 1:
                            carry = finish
                        else:
                            finish()
                # drain leftover fillers for this qc
                for f in fillers:
                    if f is not None:
                        f()

            attention(0, fillers_qc0())
            attention(1, fillers_qc1())
            with nc.named_scope("tail"):
                for qt in range(ST // QC, ST):
                    for jc in range(QC):
                        outproj_unit(1, qt, jc)

    nc.compile()
    return nc


def _get_nc():
    if "nc" not in _STATE:
        _STATE["nc"] = _build()
    return _STATE["nc"]


def _sel_const():
    sel = np.zeros((2, P), np.float32)
    sel[0, 0:DH] = 1.0
    sel[1, DH:P] = 1.0
    return sel


def _in_maps(x, mask, q_w, q_b, k_w, k_b, v_w, out_w):
    import ml_dtypes
    f = np.float32
    bf16 = ml_dtypes.bfloat16
    maps = []
    for c in range(8):
        b, g = divmod(c, 2)
        fs = slice(g * FPC, (g + 1) * FPC)
        maps.append({
            "xt": np.ascontiguousarray(x[b].T).astype(bf16),
            "wqt": np.ascontiguousarray(q_w[fs, :].T).astype(bf16),
            "wkt": np.ascontiguousarray(k_w[fs, :].T).astype(bf16),
            "wvt": np.ascontiguousarray(v_w[fs, :].T).astype(bf16),
            "wot": np.ascontiguousarray(out_w[:, fs].T).astype(bf16),
            "qb": np.ascontiguousarray(
                (q_b[fs].astype(f) / 8.0).reshape(FT, P).T),
            "kb": np.ascontiguousarray(k_b[fs].astype(f).reshape(FT, P).T),
            "negb": np.ascontiguousarray(
                np.where(mask[b] == 0, f(-30000.0), f(0.0)).astype(f)
                .reshape(ST, P).T),
            "sel": _sel_const(),
        })
    return maps


def kernel(x, mask, q_w, q_b, k_w, k_b, v_w, v_b, out_w, out_b):
    global LAST_RESULT
    from concourse import bass_utils

    x = np.asarray(x, np.float32)
    mask = np.asarray(mask)
    nc = _get_nc()
    q_w = np.asarray(q_w, np.float32)
    k_w = np.asarray(k_w, np.float32)
    v_w = np.asarray(v_w, np.float32)
    out_w = np.asarray(out_w, np.float32)
    v_b = np.asarray(v_b, np.float32)
    maps = _in_maps(x, mask, q_w, np.asarray(q_b, np.float32), k_w,
                    np.asarray(k_b, np.float32), v_w, out_w)
    res = bass_utils.run_bass_kernel_spmd(
        nc, maps, core_ids=list(range(8)), trace=TRACE,
        trace_kwargs=TRACE_KWARGS)
    LAST_RESULT = res
    out_b = np.asarray(out_b, np.float32)
    # v-bias folded through the out projection (exact under softmax norm)
    bias = out_b + out_w @ v_b
    full = np.empty((BS, SLEN, DIM), np.float32)
    for b in range(BS):
        full[b] = res.results[2 * b]["out"] + res.results[2 * b + 1]["out"] + bias
    return full


# revision 31
# speedup vs baseline: 1.0668x; 1.0668x over previous
"""Multi-head attention Trainium2 kernel (bs=4, slen=1024, dim=1024, 16 heads).

Sharding: 8 cores = 4 batches x 2 head-groups (8 heads / 512 features each).

v3 design (power-throttle aware, bf16 matmuls):
  - All matmuls bf16 (1 cyc/col on the PE; fp8 DoubleRow was tried and is
    numerically unusable here: the softmax is near-uniform so the context
    is an incoherent average and per-element fp8 noise lands on the output
    at full relative strength, ~3 percent).
  - ScalarE runs ONLY the softmax exps: 64 x [128, 2x512] batched over the
    two heads of a pair sharing a 2-bank PSUM tile, writing bf16 weights,
    plus the 8 partition-crossing ctx stashes nothing else can do.
  - PSUM->SBUF copies on DVE; softmax denominators hop partitions via
    SBUF->SBUF DMA; one DVE reciprocal per seq chunk; 1/den broadcast by a
    selector matmul.
  - Static interleave: the scores stream is padded with proj/v/ctx/outproj
    matmuls so the PE never waits on ScalarE exp latency; out-projection of
    chunk 0 overlaps attention of chunk 1.
  - v bias is dropped on-device; host folds v_b @ out_w.T into the output
    bias (exact under softmax normalization).
Host sums the two head-group partials per batch and adds the biases.
"""

import numpy as np

BS, SLEN, DIM = 4, 1024, 1024
H, DH = 16, 64
P = 128            # partitions
NB = 512           # matmul free-dim chunk (one PSUM bank of fp32)
FPC = 512          # features per core (8 heads)
DT = DIM // P      # 8 contraction tiles over model dim
FT = FPC // P      # 4 feature tiles per core
QC = SLEN // NB    # 2 seq chunks
ST = SLEN // P     # 8 seq tiles
HP = 4             # head pairs per core

_STATE = {}

# set to True by test harness to capture an NTFF profile
TRACE = False
TRACE_KWARGS = {}
LAST_RESULT = None


def _build():
    from contextlib import ExitStack

    import concourse.tile as tile
    from concourse import bacc, mybir

    f32 = mybir.dt.float32
    f32r = mybir.dt.float32r
    bf16 = mybir.dt.bfloat16
    AF = mybir.ActivationFunctionType
    ALU = mybir.AluOpType

    nc = bacc.Bacc("TRN2", target_bir_lowering=False, debug=False)

    xt_d = nc.dram_tensor("xt", [DIM, SLEN], bf16, kind="ExternalInput")
    wqt_d = nc.dram_tensor("wqt", [DIM, FPC], bf16, kind="ExternalInput")
    wkt_d = nc.dram_tensor("wkt", [DIM, FPC], bf16, kind="ExternalInput")
    wvt_d = nc.dram_tensor("wvt", [DIM, FPC], bf16, kind="ExternalInput")
    wot_d = nc.dram_tensor("wot", [FPC, DIM], bf16, kind="ExternalInput")
    qb_d = nc.dram_tensor("qb", [P, FT], f32, kind="ExternalInput")
    kb_d = nc.dram_tensor("kb", [P, FT], f32, kind="ExternalInput")
    negb_d = nc.dram_tensor("negb", [P, ST], f32, kind="ExternalInput")
    sel_d = nc.dram_tensor("sel", [2 * HP, HP, P], f32r, kind="ExternalInput")
    out_d = nc.dram_tensor("out", [SLEN, DIM], f32, kind="ExternalOutput")

    with tile.TileContext(nc) as tc:
        with ExitStack() as ctx:
            consts = ctx.enter_context(tc.tile_pool(name="consts", bufs=1))
            big = ctx.enter_context(tc.tile_pool(name="big", bufs=1))
            sm = ctx.enter_context(tc.tile_pool(name="sm", bufs=1))
            psum = ctx.enter_context(tc.tile_pool(name="psum", bufs=1, space="PSUM"))

            # ---- constants / small inputs ----
            qb_sb = consts.tile([P, FT], f32)
            nc.sync.dma_start(qb_sb, qb_d[:])
            kb_sb = consts.tile([P, FT], f32)
            nc.sync.dma_start(kb_sb, kb_d[:])
            negb_sb = consts.tile([P, ST], f32)
            nc.sync.dma_start(negb_sb, negb_d[:])
            sel_sb = consts.tile([2 * HP, HP, P], f32r)
            nc.sync.dma_start(sel_sb, sel_d[:])

            # ---- big SBUF tensors ----
            xt_sb = big.tile([P, DT, SLEN], bf16, name="xt")
            wqt_sb = big.tile([P, DT, FPC], bf16, name="wqt")
            wkt_sb = big.tile([P, DT, FPC], bf16, name="wkt")
            wvt_sb = big.tile([P, DT, FPC], bf16, name="wvt")
            wot_sb = big.tile([P, FT, DIM], bf16, name="wot")
            qT_sb = big.tile([P, FT, SLEN], bf16, name="qT")
            kT_sb = big.tile([P, FT, SLEN], bf16, name="kT")
            v_sb = big.tile([P, ST, HP * 2, DH + 1], bf16, name="v")
            wts2 = big.tile([P, ST, 2, NB], bf16, name="wts")
            ctall = big.tile([P, HP * QC, NB], f32, name="ctall")
            ctxn_sb = big.tile([P, HP, SLEN], bf16, name="ctxn")
            # denominator staging at partition 64 (psum-row aligned), then
            # DMA-gathered to partitions 0-7 for the reciprocal
            dnst = big.tile([DH + 1, 2 * HP, NB], f32, name="dnst")
            dnl = [big.tile([2 * HP, NB], f32, name=f"dnl{q}") for q in range(QC)]
            rca = [big.tile([2 * HP, NB], f32r, name=f"rca{q}") for q in range(QC)]

            # input DMAs in consumption order: x/wq interleaved (the first
            # projection chain consumes them t-by-t), then wk, wv, wo
            for t in range(DT):
                nc.sync.dma_start(xt_sb[:, t, :], xt_d[t * P:(t + 1) * P, :])
                nc.sync.dma_start(wqt_sb[:, t, :], wqt_d[t * P:(t + 1) * P, :])
            for t in range(DT):
                nc.sync.dma_start(wkt_sb[:, t, :], wkt_d[t * P:(t + 1) * P, :])
            for t in range(DT):
                nc.sync.dma_start(wvt_sb[:, t, :], wvt_d[t * P:(t + 1) * P, :])
            for t in range(FT):
                nc.sync.dma_start(wot_sb[:, t, :], wot_d[t * P:(t + 1) * P, :])

            # denominator ones-column of v_aug
            nc.vector.memset(v_sb[:, :, :, DH:DH + 1], 1.0)

            # ---------- emission units ----------
            def proj_unit(dst, wt_sb, ft, qc, scale, bias_ap):
                """x.T @ W.T chunk -> dst[:, ft, qc*NB:...]."""
                sl = slice(qc * NB, (qc + 1) * NB)
                ps = psum.tile([P, NB], f32, tag="mmo", bufs=2, name="mmo")
                for t in range(DT):
                    nc.tensor.matmul(
                        ps,
                        lhsT=wt_sb[:, t, ft * P:(ft + 1) * P],
                        rhs=xt_sb[:, t, sl],
                        start=(t == 0), stop=(t == DT - 1))
                nc.vector.tensor_scalar(
                    out=dst[:, ft, sl], in0=ps, scalar1=scale,
                    scalar2=bias_ap, op0=ALU.mult, op1=ALU.add)

            def v_unit(st):
                """v (seq-major) tile st."""
                ps = psum.tile([P, NB], f32, tag="mmo", bufs=2, name="mmo")
                for t in range(DT):
                    nc.tensor.matmul(
                        ps,
                        lhsT=xt_sb[:, t, st * P:(st + 1) * P],
                        rhs=wvt_sb[:, t, :],
                        start=(t == 0), stop=(t == DT - 1))
                nc.vector.tensor_copy(
                    v_sb[:, st, :, 0:DH],
                    ps.rearrange("p (h e) -> p h e", h=HP * 2))

            def sc_unit(qc, hp, kt):
                """scoresT for both heads of pair hp, key tile kt; exp->bf16."""
                sl = slice(qc * NB, (qc + 1) * NB)
                ksl = slice(kt * P, (kt + 1) * P)
                sc = psum.tile([P, 2, NB], f32, tag="sc", bufs=2, name="sc")
                nc.tensor.matmul(
                    sc[:, 0, :], lhsT=kT_sb[0:DH, hp, ksl],
                    rhs=qT_sb[0:DH, hp, sl], tile_position=(0, 0))
                nc.tensor.matmul(
                    sc[:, 1, :], lhsT=kT_sb[DH:P, hp, ksl],
                    rhs=qT_sb[DH:P, hp, sl], tile_position=(DH, 0))
                nc.scalar.activation(
                    wts2[:, kt, :, :], sc, AF.Exp,
                    bias=negb_sb[:, kt:kt + 1], scale=1.0)

            def ctx_mm(pc, hp, a, kt):
                """ctx accumulation for head a of pair hp, key tile kt."""
                nc.tensor.matmul(
                    pc, lhsT=v_sb[:, kt, 2 * hp + a, :],
                    rhs=wts2[:, kt, a, :],
                    start=(kt == 0), stop=(kt == ST - 1))

            def stash_unit(qc, hp, a, pc):
                """ctx rows -> ctall; denominator row -> dnst -> dnl via DMA.

                Head a=0 rows are partition-aligned (DVE); head a=1 rows must
                cross partitions 0-63 -> 64-127, which only ScalarE can do.
                The denominator hops partitions via an SBUF->SBUF DMA.
                """
                j = hp * QC + qc
                if a == 0:
                    nc.vector.tensor_copy(ctall[0:DH, j, :], pc[0:DH, :])
                else:
                    nc.scalar.copy(ctall[DH:P, j, :], pc[0:DH, :])
                r = 2 * hp + a
                nc.vector.tensor_copy(
                    dnst[DH:DH + 1, r, :], pc[DH:DH + 1, :])
                nc.sync.dma_start(
                    dnl[qc][r:r + 1, :], dnst[DH:DH + 1, r, :])

            def recip_unit(qc):
                with nc.allow_low_precision(reason="softmax recip"):
                    nc.vector.reciprocal(rca[qc][:], dnl[qc][:])

            def norm_unit(qc, hp):
                """broadcast 1/den via selector matmul; multiply on DVE."""
                sl = slice(qc * NB, (qc + 1) * NB)
                j = hp * QC + qc
                pb = psum.tile([P, NB], f32, tag="mmo", bufs=2, name="mmo")
                nc.tensor.matmul(pb, lhsT=sel_sb[:, hp, :], rhs=rca[qc][:])
                nc.vector.tensor_mul(ctxn_sb[:, hp, sl], ctall[:, j, :], pb)

            def outproj_unit(qc, qt, jc):
                po = psum.tile([P, NB], f32, tag="mmo", bufs=2, name="mmo")
                for ft in range(FT):
                    nc.tensor.matmul(
                        po, lhsT=ctxn_sb[:, ft, qt * P:(qt + 1) * P],
                        rhs=wot_sb[:, ft, jc * NB:(jc + 1) * NB],
                        start=(ft == 0), stop=(ft == FT - 1))
                ob = sm.tile([P, NB], f32, tag="outsb", bufs=4, name="ob")
                nc.vector.tensor_copy(ob, po)
                nc.sync.dma_start(
                    out_d[qt * P:(qt + 1) * P, jc * NB:(jc + 1) * NB], ob)

            # ---------- static schedule ----------
            def qf(ft, qc):
                return lambda: proj_unit(
                    qT_sb, wqt_sb, ft, qc, 0.125, qb_sb[:, ft:ft + 1])

            def kf(ft, qc):
                return lambda: proj_unit(
                    kT_sb, wkt_sb, ft, qc, 1.0, kb_sb[:, ft:ft + 1])

            # prologue: everything block hp0 of qc0 needs up front
            with nc.named_scope("prologue"):
                qf(0, 0)()
                kf(0, 0)()
                kf(0, 1)()
                v_unit(0)
                v_unit(1)
                v_unit(6)
                v_unit(7)

            def fillers_qc0():
                # 7 slots per hp block; constraints: v_kt emitted before
                # hp0's ctx for that kt; k ft (both qc) and q(ft, qc0)
                # complete before block ft's first scores; q(ft, qc1)
                # any time before qc1 attention.
                units = [
                    # hp0 slots
                    lambda: v_unit(2), lambda: v_unit(3), lambda: v_unit(4),
                    lambda: v_unit(5), qf(1, 0), kf(1, 0), kf(1, 1),
                    # hp1 slots
                    qf(2, 0), kf(2, 0), kf(2, 1), qf(0, 1), qf(1, 1),
                    None, None,
                    # hp2 slots
                    qf(3, 0), kf(3, 0), kf(3, 1), qf(2, 1), None, None, None,
                    # hp3 slots
                    qf(3, 1),
                ]
                yield from units

            def fillers_qc1():
                for hp in range(HP):
                    yield lambda hp=hp: norm_unit(0, hp)
                for qt in range(ST // QC):
                    for jc in range(QC):
                        yield lambda qt=qt, jc=jc: outproj_unit(0, qt, jc)

            def attention(qc, fillers):
                def slot():
                    f = next(fillers, None)
                    if f is not None:
                        f()

                carry = None  # deferred (ctx kt 5-7 + stash) of previous hp
                for hp in range(HP):
                    with nc.named_scope("attn"):
                        pcA = psum.tile([DH + 1, NB], f32, tag="ctx", bufs=2,
                                        name="pcA")
                        pcB = psum.tile([DH + 1, NB], f32, tag="ctx", bufs=2,
                                        name="pcB")
                        sc_unit(qc, hp, 0)
                        if carry is not None:
                            carry()
                            carry = None
                        sc_unit(qc, hp, 1)
                        slot()
                        sc_unit(qc, hp, 2)
                        slot()
                        sc_unit(qc, hp, 3)
                        slot()
                        ctx_mm(pcA, hp, 0, 0)
                        ctx_mm(pcB, hp, 1, 0)
                        sc_unit(qc, hp, 4)
                        slot()
                        ctx_mm(pcA, hp, 0, 1)
                        ctx_mm(pcB, hp, 1, 1)
                        sc_unit(qc, hp, 5)
                        slot()
                        ctx_mm(pcA, hp, 0, 2)
                        ctx_mm(pcB, hp, 1, 2)
                        sc_unit(qc, hp, 6)
                        slot()
                        ctx_mm(pcA, hp, 0, 3)
                        ctx_mm(pcB, hp, 1, 3)
                        sc_unit(qc, hp, 7)
                        slot()
                        ctx_mm(pcA, hp, 0, 4)
                        ctx_mm(pcB, hp, 1, 4)

                        def finish(qc=qc, hp=hp, pcA=pcA, pcB=pcB):
                            for kt in (5, 6, 7):
                                ctx_mm(pcA, hp, 0, kt)
                                ctx_mm(pcB, hp, 1, kt)
                            stash_unit(qc, hp, 0, pcA)
                            stash_unit(qc, hp, 1, pcB)
                        if hp < HP - 1:
                            carry = finish
                        else:
                            finish()
                # drain leftover fillers for this qc
                for f in fillers:
                    if f is not None:
                        f()

            attention(0, fillers_qc0())
            recip_unit(0)
            attention(1, fillers_qc1())
            recip_unit(1)
            with nc.named_scope("tail"):
                for hp in range(HP):
                    norm_unit(1, hp)
                for qt in range(ST // QC, ST):
                    for jc in range(QC):
                        outproj_unit(1, qt, jc)

    nc.compile()
    return nc


def _get_nc():
    if "nc" not in _STATE:
        _STATE["nc"] = _build()
    return _STATE["nc"]


def _sel_const():
    sel = np.zeros((2 * HP, HP, P), np.float32)
    for hp in range(HP):
        sel[2 * hp, hp, 0:DH] = 1.0
        sel[2 * hp + 1, hp, DH:P] = 1.0
    return sel


def _in_maps(x, mask, q_w, q_b, k_w, k_b, v_w, out_w):
    import ml_dtypes
    f = np.float32
    bf16 = ml_dtypes.bfloat16
    maps = []
    for c in range(8):
        b, g = divmod(c, 2)
        fs = slice(g * FPC, (g + 1) * FPC)
        maps.append({
            "xt": np.ascontiguousarray(x[b].T).astype(bf16),
            "wqt": np.ascontiguousarray(q_w[fs, :].T).astype(bf16),
            "wkt": np.ascontiguousarray(k_w[fs, :].T).astype(bf16),
            "wvt": np.ascontiguousarray(v_w[fs, :].T).astype(bf16),
            "wot": np.ascontiguousarray(out_w[:, fs].T).astype(bf16),
            "qb": np.ascontiguousarray(
                (q_b[fs].astype(f) / 8.0).reshape(FT, P).T),
            "kb": np.ascontiguousarray(k_b[fs].astype(f).reshape(FT, P).T),
            "negb": np.ascontiguousarray(
                np.where(mask[b] == 0, f(-30000.0), f(0.0)).astype(f)
                .reshape(ST, P).T),
            "sel": _sel_const(),
        })
    return maps


def kernel(x, mask, q_w, q_b, k_w, k_b, v_w, v_b, out_w, out_b):
    global LAST_RESULT
    from concourse import bass_utils

    x = np.asarray(x, np.float32)
    mask = np.asarray(mask)
    nc = _get_nc()
    q_w = np.asarray(q_w, np.float32)
    k_w = np.asarray(k_w, np.float32)
    v_w = np.asarray(v_w, np.float32)
    out_w = np.asarray(out_w, np.float32)
    v_b = np.asarray(v_b, np.float32)
    maps = _in_maps(x, mask, q_w, np.asarray(q_b, np.float32), k_w,
                    np.asarray(k_b, np.float32), v_w, out_w)
    res = bass_utils.run_bass_kernel_spmd(
        nc, maps, core_ids=list(range(8)), trace=TRACE,
        trace_kwargs=TRACE_KWARGS)
    LAST_RESULT = res
    out_b = np.asarray(out_b, np.float32)
    # v-bias folded through the out projection (exact under softmax norm)
    bias = out_b + out_w @ v_b
    full = np.empty((BS, SLEN, DIM), np.float32)
    for b in range(BS):
        full[b] = res.results[2 * b]["out"] + res.results[2 * b + 1]["out"] + bias
    return full
